# revision 30
# baseline (speedup 1.0000x reference)
"""nn_Attention Trainium2 Bass kernel (restructured, bf16).

Full attention forward: x->(q,k,v) with l2-normalized weights, per-head-dim
l2 norm + learned qk scale, interleaved RoPE, causal SDPA, output projection
with column-l2-normalized wo.

Sharding: TP=4 over heads (8 heads/core) x DP=2 over batch across 8 cores.
Each core computes a partial [2048, 2048] output (bf16) for its batch; host
sums the 4 TP partials per batch in f32.

Device structure (single TileContext, software-pipelined):
  proj(st=0); then attn(0)<-proj(1), attn(1)<-proj(2), attn(2)<-proj(3)+
  yproj(0), attn(3)<-yproj(1,2); yproj(3) last. Fillers are generators
  whose PE chunks are interleaved into the attention stream to cover the
  exp (scalar-engine) latency and the DVE post-processing chains.
- x streamed per 512-row block (double buffered) instead of fully resident.
- rope-pair permutation folded into wq/wk rows host-side; qk_scale folded
  into cos/sin tables; rope runs on bf16 SBUF tiles (DVE fast path), the
  rotate-half is expressed as two strided-view multiplies (no copies).
- transposed softmax: logitsT [sj, si]; exp without max subtraction
  (|logit| <= qk_scale^2 = 8); causal masking via -1e30 adds into the
  logits PSUM on the two diagonal 256-col pairs only; the second diagonal
  pair computes only its valid 256-col range.
- softmax denominators ride as row 64 of the PV psum (ones column in v);
  per-head normalize uses reciprocal_approx_fast + a K=1 broadcast matmul.
- attention output overwrites qT storage; yproj writes bf16 Y.
"""
import sys
import os
from contextlib import ExitStack

sys.path.insert(0, "/opt/trn_rl_repo")

import numpy as np
import ml_dtypes

BF16 = ml_dtypes.bfloat16

B, S, DIM = 2, 2048, 2048
HEADS, DH = 32, 64
THETA = 10000.0
NCORES = 8
TP = 4             # head-parallel ways
HPC = HEADS // TP  # heads per core = 8
E = HPC * DH       # per-core qkv width = 512
ET = E // 128      # e-tiles per core = 4
DT = DIM // 128    # contraction d-tiles = 16
SB = S // 512      # 512-wide seq blocks = 4
SS = S // 128      # 128-wide seq blocks = 16

_CACHE = {}


def _l2n(w, axis):
    n = np.sqrt((w.astype(np.float64) ** 2).sum(axis=axis, keepdims=True))
    n = np.maximum(n, 1e-12)
    return (w / n).astype(np.float32)


def _build_program():
    import concourse.bass as bass
    from concourse import bacc
    import concourse.mybir as mybir
    import concourse.tile as tile

    f32 = mybir.dt.float32
    bf16 = mybir.dt.bfloat16
    AF = mybir.ActivationFunctionType
    AX = mybir.AxisListType
    OP = mybir.AluOpType

    nc = bacc.Bacc("TRN2", target_bir_lowering=False)

    f32d = f32  # mask tiles stay f32 (match psum dtype for DVE adds)
    xT = nc.dram_tensor("xT", [128, SB, DT, 512], bf16, kind="ExternalInput")
    wqT = nc.dram_tensor("wqT", [128, DT, E], bf16, kind="ExternalInput")
    wkT = nc.dram_tensor("wkT", [128, DT, E], bf16, kind="ExternalInput")
    wvT = nc.dram_tensor("wvT", [128, DT, E], bf16, kind="ExternalInput")
    woT = nc.dram_tensor("woT", [128, ET, DIM], bf16, kind="ExternalInput")
    cosd = nc.dram_tensor("cosd", [128, SS * DH], bf16, kind="ExternalInput")
    sind = nc.dram_tensor("sind", [128, SS * DH], bf16, kind="ExternalInput")
    mtrid = nc.dram_tensor("mtrid", [128, 128], f32d, kind="ExternalInput")
    Y = nc.dram_tensor("Y", [S, DIM], bf16, kind="ExternalOutput")

    with tile.TileContext(nc) as tc, ExitStack() as ctx:
        const = ctx.enter_context(tc.tile_pool(name="const", bufs=1))
        wpool = ctx.enter_context(tc.tile_pool(name="wpool", bufs=1))
        xpool = ctx.enter_context(tc.tile_pool(name="xpool", bufs=3))
        qkv = ctx.enter_context(tc.tile_pool(name="qkv", bufs=1))
        work = ctx.enter_context(tc.tile_pool(name="work", bufs=1))
        prp = ctx.enter_context(tc.tile_pool(name="prp", bufs=2, space="PSUM"))
        lgp = ctx.enter_context(tc.tile_pool(name="lgp", bufs=2, space="PSUM"))
        pvp = ctx.enter_context(tc.tile_pool(name="pvp", bufs=2, space="PSUM"))

        # constants
        cos_sb = const.tile([128, SS, DH], bf16)
        sin_sb = const.tile([128, SS, DH], bf16)
        mtri_sb = const.tile([128, 128], f32)

        ones_t = const.tile([128, 64], bf16)
        nc.vector.memset(ones_t, 1.0)

        # persistent activations (qT doubles as attention-output storage)
        qT = qkv.tile([128, ET, S], bf16)
        kT = qkv.tile([128, ET, S], bf16)
        v_sb = qkv.tile([128, SS, HPC, 65], bf16)
        nc.vector.memset(v_sb[:, :, :, 64:65], 1.0)

        # weights
        wq_sb = wpool.tile([128, DT, E], bf16, tag="wq")
        wk_sb = wpool.tile([128, DT, E], bf16, tag="wk")
        wv_sb = wpool.tile([128, DT, E], bf16, tag="wv")
        wo_sb = wpool.tile([128, ET, DIM], bf16, tag="wo")

        x_tiles = {}

        def load_x(st):
            t = xpool.tile([128, DT, 512], bf16, tag="x", name=f"x{st}")
            for g in range(0, DT, 4):
                nc.sync.dma_start(t[:, g:g + 4, :], xT[:, st, g:g + 4, :])
            x_tiles[st] = t

        qb_tiles = {}
        ssq_tiles = {}

        def qk_postA(kind, st, su, ps):
            """psum [si,e] -> bf16 copy + per-head square-sum accumulation."""
            qb = work.tile([128, E], bf16, tag=f"qb{kind}", bufs=4,
                           name=f"qb{kind}_{su}")
            nc.scalar.copy(qb, ps)
            qb_tiles[(kind, su)] = qb
            sq = work.tile([128, E], bf16, tag="sq", bufs=2)
            nc.vector.tensor_mul(sq, qb, qb)
            if su == 0:
                ssq_tiles[kind] = work.tile(
                    [128, 4, HPC], f32, tag=f"ssq{kind}", bufs=2, name=f"ssq{kind}")
            nc.vector.tensor_reduce(
                ssq_tiles[kind][:, su, :],
                sq.rearrange("p (h d) -> p h d", d=DH), axis=AX.X, op=OP.add)

        def qk_postB(kind, st):
            """batched 1/|q| for 4 su, then rope + transposed store per su."""
            ssq = ssq_tiles.pop(kind)
            nrm = work.tile([128, 4, HPC], f32, tag="nrm", bufs=2)
            nc.scalar.sqrt(nrm, ssq)
            inv = work.tile([128, 4, HPC], f32, tag="inv", bufs=2)
            nc.vector.reciprocal(inv, nrm)
            dstT = qT if kind == 0 else kT
            for su in range(4):
                sblk = st * 4 + su
                qb = qb_tiles.pop((kind, su))
                qn = work.tile([128, HPC, DH], bf16, tag="qn", bufs=2)
                nc.vector.tensor_mul(
                    qn, qb.rearrange("p (h d) -> p h d", d=DH),
                    inv[:, su, :].unsqueeze(2).broadcast_to([128, HPC, DH]))
                qn4 = qn.rearrange("p h (t u) -> p h t u", u=32)
                rot = work.tile([128, HPC, 2, 32], bf16, tag="rot", bufs=2)
                nc.vector.tensor_mul(
                    rot[:, :, 0, :], qn4[:, :, 1, :],
                    sin_sb[:, sblk:sblk + 1, 0:32].broadcast_to([128, HPC, 32]))
                nc.vector.tensor_mul(
                    rot[:, :, 1, :], qn4[:, :, 0, :],
                    sin_sb[:, sblk:sblk + 1, 32:64].broadcast_to([128, HPC, 32]))
                qf = work.tile([128, HPC, DH], bf16, tag="qf", bufs=2)
                nc.vector.tensor_mul(
                    qf, qn, cos_sb[:, sblk:sblk + 1, :].broadcast_to([128, HPC, DH]))
                qo = work.tile([128, E], bf16, tag="qo", bufs=2)
                nc.vector.tensor_add(
                    qo, qf.rearrange("p h d -> p (h d)"),
                    rot.rearrange("p h t u -> p (h t u)"))
                nc.sync.dma_start_transpose(
                    dstT[:, :, sblk * 128:(sblk + 1) * 128], qo)

        def v_post(st, su, ps):
            sblk = st * 4 + su
            nc.scalar.copy(
                v_sb[:, sblk, :, 0:64],
                ps.rearrange("p (h d) -> p h d", d=DH))

        def proj_gen(st):
            """Generator: emits proj for st; yields after each PE chunk."""
            for stt in (st, st + 1):
                if stt < SB and stt not in x_tiles:
                    load_x(stt)
            xt = x_tiles[st]
            for kind, w_sb in enumerate((wq_sb, wk_sb, wv_sb)):
                for su in range(4):
                    ps = prp.tile([128, E], f32, tag="pr", name=f"ps{st}_{kind}_{su}")
                    for g in range(4):
                        for dt in range(g * 4, g * 4 + 4):
                            nc.tensor.matmul(
                                ps,
                                xt[:, dt, su * 128:(su + 1) * 128],
                                w_sb[:, dt, :],
                                start=(dt == 0), stop=(dt == DT - 1))
                        yield
                    if kind == 2:
                        v_post(st, su, ps)
                    else:
                        qk_postA(kind, st, su, ps)
                        if su == 3:
                            qk_postB(kind, st)
                    yield
            del x_tiles[st]

        def normalize_head(h, i, dns):
            et, hp = h // 2, (h % 2) * 64
            bp = 32 * (h % 3)
            bc = lgp.tile([128, 512], f32, tag="lg", name=f"bc{h}")
            nc.tensor.matmul(bc[hp:hp + 64, :], ones_t[bp:bp + 1, :],
                             dns[bp:bp + 1, h // 3, :],
                             start=True, stop=True)
            bcsb = work.tile([128, 512], bf16, tag="bcs", bufs=2)
            nc.scalar.copy(bcsb[hp:hp + 64, :], bc[hp:hp + 64, :])
            sl = qT[hp:hp + 64, et, i * 512:(i + 1) * 512]
            nc.vector.tensor_mul(sl, sl, bcsb[hp:hp + 64, :])

        def attn_block(i, fill):
            npr = 2 * (i + 1)
            dns = work.tile([128, 3, 512], bf16, tag="dns", bufs=1, name=f"dns{i}")
            nc.vector.memset(dns, 1.0)
            for h in range(HPC):
                et, hp = h // 2, (h % 2) * 64
                pv = pvp.tile([128, 512], f32, tag="pv", name=f"pv{i}_{h}")
                lgs = {}

                def emit_lg(p):
                    # per-sjb valid si-column start: c0 = 128*r for diagonal
                    # offsets r >= 0; exp of masked cols is skipped entirely
                    lg2 = lgp.tile([128, 2, 512], f32, tag="lg", name=f"lg{p}")
                    for b in range(2):
                        sjb = 2 * p + b
                        r = sjb - 4 * i
                        c0 = 128 * r if r > 0 else 0
                        nc.tensor.matmul(
                            lg2[:, b, c0:512],
                            kT[hp:hp + 64, et, sjb * 128:(sjb + 1) * 128],
                            qT[hp:hp + 64, et, i * 512 + c0:(i + 1) * 512],
                            start=True, stop=True)
                        if r >= 0:
                            nc.vector.tensor_add(
                                lg2[:, b, 128 * r:128 * r + 128],
                                lg2[:, b, 128 * r:128 * r + 128], mtri_sb)
                    lgs[p] = lg2

                emit_lg(0)
                if npr > 1:
                    emit_lg(1)
                for p in range(npr):
                    lg2 = lgs.pop(p)
                    ex = work.tile([128, 2, 512], bf16, tag="ex", bufs=3)
                    diag = (p >= npr - 2)
                    if diag:
                        for b in range(2):
                            r = 2 * p + b - 4 * i
                            c0 = 128 * r if r > 0 else 0
                            nc.scalar.activation(
                                ex[:, b, c0:512], lg2[:, b, c0:512], AF.Exp)
                    else:
                        nc.scalar.activation(ex, lg2, AF.Exp)
                    if p + 2 < npr:
                        emit_lg(p + 2)
                    fill(2 if h < 4 else 1)
                    for b in range(2):
                        sjb = 2 * p + b
                        r = sjb - 4 * i
                        c0 = 128 * r if r > 0 else 0
                        nc.tensor.matmul(
                            pv[0:65, c0:512],
                            v_sb[:, sjb, h, :],
                            ex[:, b, c0:512],
                            start=(sjb == 0),
                            stop=(p == npr - 1 and b == 1),
                            skip_group_check=True)
                # stash denominator row + unnormalized outT; divide later
                nc.vector.tensor_copy(
                    dns[32 * (h % 3):32 * (h % 3) + 1, h // 3, :], pv[64:65, :])
                nc.vector.tensor_copy(
                    qT[hp:hp + 64, et, i * 512:(i + 1) * 512], pv[0:64, :])
                if h in (2, 5, 7):
                    c = h // 3
                    with nc.allow_low_precision("bf16 softmax denominators"):
                        nc.vector.reciprocal(dns[:, c, :], dns[:, c, :])
                fill()
            for h in range(HPC):
                normalize_head(h, i, dns)
                fill()

        def yproj_gen(i, on_dve=False):
            for ib in range(4 * i, 4 * i + 4):
                ys = work.tile([128, 4, 512], bf16, tag="ys", bufs=2)
                for nd in range(4):
                    yps = prp.tile([128, 512], f32, tag="pr", name=f"yps{nd}")
                    for ket in range(ET):
                        nc.tensor.matmul(
                            yps,
                            qT[:, ket, ib * 128:(ib + 1) * 128],
                            wo_sb[:, ket, nd * 512:(nd + 1) * 512],
                            start=(ket == 0), stop=(ket == ET - 1))
                    if on_dve:
                        nc.vector.tensor_copy(ys[:, nd, :], yps)
                    else:
                        nc.scalar.copy(ys[:, nd, :], yps)
                    yield
                for half in range(2):
                    nc.sync.dma_start(
                        Y[ib * 128:(ib + 1) * 128,
                          half * 1024:(half + 1) * 1024],
                        ys.rearrange("p n s -> p (n s)")
                        [:, half * 1024:(half + 1) * 1024])

        # ---- emission schedule ----
        for g in range(0, DT, 4):
            nc.scalar.dma_start(wq_sb[:, g:g + 4, :], wqT[:, g:g + 4, :])
        gen0 = proj_gen(0)
        for k in range(4):
            next(gen0)  # x(0) dma + first q chunks
        nc.scalar.dma_start(cos_sb, cosd.rearrange("p (b d) -> p b d", d=DH))
        nc.scalar.dma_start(sin_sb, sind.rearrange("p (b d) -> p b d", d=DH))
        nc.scalar.dma_start(mtri_sb, mtrid[:])
        nc.scalar.dma_start(wk_sb, wkT[:])
        for k in range(6):
            next(gen0, None)
        nc.scalar.dma_start(wv_sb, wvT[:])
        for k in range(6):
            next(gen0, None)
        nc.scalar.dma_start(wo_sb, woT[:])
        for _ in gen0:
            pass

        import itertools

        for i in range(SB):
            parts = []
            if i < SB - 1:
                parts.append(proj_gen(i + 1))
            if i == 2:
                parts.append(yproj_gen(0))
            if i == 3:
                parts.append(yproj_gen(1, on_dve=True))
                parts.append(yproj_gen(2, on_dve=True))
            filler = itertools.chain(*parts)

            def fill(n=1):
                for _ in range(n):
                    next(filler, None)

            attn_block(i, fill)
            for _ in filler:
                pass
        for _ in yproj_gen(SB - 1):
            pass

    return nc


def _host_prep(x, wq, wk, wv, wo, qk_scale):
    """Returns per-core input dicts."""
    perm = np.concatenate([np.arange(0, DH, 2), np.arange(1, DH, 2)])
    wq_n = _l2n(wq, -1).reshape(HEADS, DH, DIM)[:, perm, :].reshape(HEADS * DH, DIM)
    wk_n = _l2n(wk, -1).reshape(HEADS, DH, DIM)[:, perm, :].reshape(HEADS * DH, DIM)
    wv_n = _l2n(wv, -1)
    wo_n = _l2n(wo, 0)
    sp = qk_scale.astype(np.float64)[perm]

    # rope tables with qk_scale folded in; permuted-block layout
    half = np.arange(0, DH, 2)
    freqs = 1.0 / (THETA ** (half.astype(np.float64) / DH))      # (32,)
    ang = np.arange(S, dtype=np.float64)[:, None] * freqs[None]  # (S, 32)
    cos_h, sin_h = np.cos(ang), np.sin(ang)
    cos_p = np.concatenate([cos_h, cos_h], 1)                    # (S, 64)
    sin_e = np.concatenate([-sin_h, sin_h], 1)
    cos_eff = (cos_p * sp[None, :]).astype(np.float32)
    swap_sp = np.concatenate([sp[32:], sp[:32]])
    sin_eff = (sin_e * swap_sp[None, :]).astype(np.float32)
    # device layout [128, SS*DH]: [p, b*64+c] = tbl[b*128+p, c]
    cosd = np.ascontiguousarray(
        cos_eff.reshape(SS, 128, DH).transpose(1, 0, 2).reshape(128, SS * DH))
    sind = np.ascontiguousarray(
        sin_eff.reshape(SS, 128, DH).transpose(1, 0, 2).reshape(128, SS * DH))

    # causal mask additive tiles (keep sj <= si_local)
    sj = np.arange(128)[:, None]
    si = np.arange(128)[None, :]
    mtri = np.where(sj <= si, 0.0, -1e30).astype(np.float32)

    in_maps = []
    for c in range(NCORES):
        b, t = divmod(c, TP)
        e0 = t * E
        xb = x[b].T.reshape(DT, 128, SB, 512).transpose(1, 2, 0, 3)
        wqb = wq_n[e0:e0 + E].T.reshape(DT, 128, E).transpose(1, 0, 2)
        wkb = wk_n[e0:e0 + E].T.reshape(DT, 128, E).transpose(1, 0, 2)
        wvb = wv_n[e0:e0 + E].T.reshape(DT, 128, E).transpose(1, 0, 2)
        wob = wo_n[:, e0:e0 + E].T.reshape(ET, 128, DIM).transpose(1, 0, 2)
        in_maps.append({
            "xT": np.ascontiguousarray(xb).astype(BF16),
            "wqT": np.ascontiguousarray(wqb).astype(BF16),
            "wkT": np.ascontiguousarray(wkb).astype(BF16),
            "wvT": np.ascontiguousarray(wvb).astype(BF16),
            "woT": np.ascontiguousarray(wob).astype(BF16),
            "cosd": cosd.astype(BF16), "sind": sind.astype(BF16),
            "mtrid": mtri,
        })
    return in_maps


def _install_profile_hook():
    """antenv.axon_hooks is absent in this image; shim it and register the
    ctypes NTFF hook against /opt/axon/libaxon_pjrt.so (mirrors trn_boot)."""
    import types
    import ctypes
    import contextlib

    try:
        from antenv.axon_hooks import get_axon_ntff_profile_hook  # noqa
        return
    except ImportError:
        pass
    import antenv
    mod = types.ModuleType("antenv.axon_hooks")
    state = {}
    mod.set_axon_ntff_profile_hook = lambda h: state.__setitem__("h", h)
    mod.get_axon_ntff_profile_hook = lambda: state.get("h")
    sys.modules["antenv.axon_hooks"] = mod
    antenv.axon_hooks = mod

    so_path = "/opt/axon/libaxon_pjrt.so"
    lib = ctypes.CDLL(so_path)
    if not hasattr(lib, "axon_start_nrt_profile"):
        return
    lib.axon_start_nrt_profile.argtypes = [
        ctypes.POINTER(ctypes.c_int64), ctypes.c_size_t]
    lib.axon_start_nrt_profile.restype = ctypes.c_int64
    lib.axon_stop_nrt_profile.argtypes = [ctypes.c_char_p]
    lib.axon_stop_nrt_profile.restype = ctypes.c_int64

    @contextlib.contextmanager
    def _hook(output_dir, device_ids):
        import jax
        jax.devices()
        if device_ids:
            ids = (ctypes.c_int64 * len(device_ids))(*device_ids)
            rc = lib.axon_start_nrt_profile(ids, len(device_ids))
        else:
            rc = lib.axon_start_nrt_profile(None, 0)
        if rc != 0:
            raise RuntimeError(f"axon_start_nrt_profile rc={rc}")
        try:
            yield
        finally:
            n = lib.axon_stop_nrt_profile(str(output_dir).encode())
            print(f"profile: {n} file(s) written to {output_dir}",
                  file=sys.stderr)

    mod.set_axon_ntff_profile_hook(_hook)


def kernel(x, wq, wk, wv, wo, qk_scale, _profile=False):
    from concourse.bass_utils import run_bass_kernel_spmd

    if _profile:
        _install_profile_hook()

    if "nc" not in _CACHE:
        nc = _build_program()
        nc.finalize()
        _CACHE["nc"] = nc
    nc = _CACHE["nc"]
    in_maps = _host_prep(np.asarray(x), np.asarray(wq), np.asarray(wk),
                         np.asarray(wv), np.asarray(wo), np.asarray(qk_scale))
    res = run_bass_kernel_spmd(nc, in_maps, core_ids=list(range(NCORES)),
                               trace=_profile)
    outs = res.results
    y = np.empty((B, S, DIM), dtype=np.float32)
    for b in range(B):
        y[b] = sum(outs[b * TP + t]["Y"].astype(np.float32) for t in range(TP))
    if _profile:
        _CACHE["last_exec_time_ns"] = res.exec_time_ns
        _CACHE["last_profile"] = res.profile_json
    return y
